# revision 1
# baseline (speedup 1.0000x reference)
"""DeepseekV2 MLA attention for 8 TRN2 NeuronCores (Bass/Tile).

Sharding: core c handles batch b=c//4, head-group g=c%4 (4 of 16 heads).
The q_a/kv_a projections + shared kv latent / k_pe are replicated within
each batch's 4 cores (MLA's point); o_proj is row-parallel with the
4 partial outputs summed on the host during the gather step.

Dataflow is fully "transposed-chain": activations live as [feature, token]
tiles so attention scores are computed directly in [k_tok, q_tok] layout
(no on-chip transposes), softmax runs without max-subtraction (logits are
O(1) by construction), and all per-token normalizations (rmsnorm scale,
softmax 1/sum) are scale rows broadcast across partitions. rotate_half is
a constant 128x128 matmul. o_proj is computed transposed ([o_dim, token])
so its weights can stream.

Matmuls run in float32r (full-rate PE mode, fp32 storage); PSUM
accumulation is fp32.
"""

import numpy as np

import concourse.bacc as bacc
import concourse.mybir as mybir
import concourse.tile as tile
from concourse.bass_utils import run_bass_kernel_spmd

F32 = mybir.dt.float32
F32R = mybir.dt.float32r
BF16 = mybir.dt.bfloat16

# problem constants
B, S, HID, QL = 2, 1024, 2048, 1536
NH, NOPE, ROPE, VD, KVL = 16, 128, 64, 128, 512
QHD = NOPE + ROPE  # 192
EPS = 1e-6
THETA = 10000.0
HG = 4          # heads per core
GW = HG * VD    # 512, attn-cat width per core
NKT = HID // 128   # 16 contraction tiles over hidden
NQL = QL // 128    # 12
HJ = S // 2        # 512 token half

MM_DT = F32R
MM_NP = np.float32


def _interleave_rows(w):
    # fold _interleave_perm into weight rows: out feature j = in feature perm[j]
    return np.concatenate([w[0::2], w[1::2]], axis=0)


def _rope_tables(positions):
    inv = 1.0 / (THETA ** (np.arange(0, ROPE, 2, dtype=np.float32) / ROPE))
    t = positions.astype(np.float32)
    freqs = np.outer(t, inv)
    emb = np.concatenate([freqs, freqs], axis=-1)  # [S, 64]
    return np.cos(emb), np.sin(emb)


def _rot_matrix():
    # R @ x = rotate_half(x) for 64-dim x; block-diag twice for 128 rows.
    R = np.zeros((ROPE, ROPE), np.float32)
    for j in range(32):
        R[j, j + 32] = -1.0
        R[j + 32, j] = 1.0
    R2 = np.zeros((128, 128), np.float32)
    R2[:64, :64] = R
    R2[64:, 64:] = R
    return R2


def prep_in_maps(inputs):
    """Full inputs -> list of 8 per-core input dicts (numpy, host-side)."""
    h = np.asarray(inputs["hidden_states"], np.float32)
    pos = np.asarray(inputs["position_ids"])
    q_a_w = np.asarray(inputs["q_a_w"], np.float32)
    q_a_ln = np.asarray(inputs["q_a_ln"], np.float32)
    q_b_w = np.asarray(inputs["q_b_w"], np.float32)
    kv_a_w = np.asarray(inputs["kv_a_w"], np.float32)
    kv_a_ln = np.asarray(inputs["kv_a_ln"], np.float32)
    kv_b_w = np.asarray(inputs["kv_b_w"], np.float32)
    o_w = np.asarray(inputs["o_w"], np.float32)

    wqa = np.ascontiguousarray(q_a_w.T).astype(MM_NP)  # [HID, QL]

    # kv_a: fold interleave perm into the k_pe rows (last 64), and duplicate
    # the pe block so k_pe^T materializes on both partition halves (heads at
    # partition base 0 and 64 both need an aligned copy).
    pe_rows_w = _interleave_rows(kv_a_w[KVL:])
    kv_a_w2 = np.concatenate([kv_a_w[:KVL], pe_rows_w, pe_rows_w], axis=0)  # [640, HID]
    wkva = np.ascontiguousarray(kv_a_w2.T).astype(MM_NP)  # [HID, 640]

    scale = QHD ** -0.5
    rot = _rot_matrix()
    rotT = np.ascontiguousarray(rot.T).astype(MM_NP)

    per_core = []
    for c in range(8):
        b, g = divmod(c, 4)
        heads = range(HG * g, HG * g + HG)

        # q_b rows for this group, blocked [4x nope(128), 2x pe-pair(128)],
        # with q_a_ln folded into columns, interleave perm folded into pe
        # rows, and the attention scale folded in.
        nope_rows = []
        pe_rows = []
        for hh in heads:
            rows = q_b_w[hh * QHD:(hh + 1) * QHD]  # [192, QL]
            nope_rows.append(rows[:NOPE])
            pe_rows.append(_interleave_rows(rows[NOPE:]))
        wqb_g = np.concatenate(nope_rows + pe_rows, axis=0)  # [768, QL]
        wqb_g = wqb_g * q_a_ln[None, :] * scale
        wqb = np.ascontiguousarray(wqb_g.T).astype(MM_NP)  # [QL, 768]

        # kv_b nope/v for this group with kv_a_ln folded
        kn_rows = []
        v_rows = []
        for hh in heads:
            rows = kv_b_w[hh * (NOPE + VD):(hh + 1) * (NOPE + VD)]
            kn_rows.append(rows[:NOPE])
            v_rows.append(rows[NOPE:])
        wkbn = np.concatenate(kn_rows, axis=0) * kv_a_ln[None, :]  # [512, KVL]
        wkbv = np.concatenate(v_rows, axis=0) * kv_a_ln[None, :]
        wkbn = np.ascontiguousarray(wkbn.T).astype(MM_NP)  # [KVL, 512]
        wkbv = np.ascontiguousarray(wkbv.T).astype(MM_NP)

        # o_w columns for this group's heads (rows of o_w.T)
        wo = np.ascontiguousarray(o_w[:, GW * g: GW * (g + 1)].T).astype(MM_NP)  # [512, HID]

        cos, sin = _rope_tables(np.asarray(pos[b]))
        cosT = np.ascontiguousarray(np.concatenate([cos.T, cos.T], axis=0))  # [128, S]
        sinT = np.ascontiguousarray(np.concatenate([sin.T, sin.T], axis=0))

        per_core.append({
            "hT": np.ascontiguousarray(h[b].T).astype(MM_NP),  # [HID, S]
            "wqa": wqa, "wqb": wqb, "wkva": wkva,
            "wkbn": wkbn, "wkbv": wkbv, "wo": wo,
            "cosT": cosT.astype(np.float32), "sinT": sinT.astype(np.float32),
            "rotT": rotT,
        })
    return per_core


def combine_outputs(results):
    """8 per-core outT [HID, S] partials -> full [B, S, HID] output."""
    out = np.zeros((B, S, HID), np.float32)
    for c, r in enumerate(results):
        b = c // 4
        out[b] += np.asarray(r["outT"]).T
    return out


def build_nc(debug=False):
    nc = bacc.Bacc("TRN2", target_bir_lowering=False, debug=False, num_devices=8)
    dram = nc.declare_dram_parameter

    hT = dram("hT", [HID, S], MM_DT, isOutput=False)
    wqa = dram("wqa", [HID, QL], MM_DT, isOutput=False)
    wqb = dram("wqb", [QL, 768], MM_DT, isOutput=False)
    wkva = dram("wkva", [HID, KVL + 2 * ROPE], MM_DT, isOutput=False)
    wkbn = dram("wkbn", [KVL, GW], MM_DT, isOutput=False)
    wkbv = dram("wkbv", [KVL, GW], MM_DT, isOutput=False)
    wo = dram("wo", [GW, HID], MM_DT, isOutput=False)
    cosT = dram("cosT", [128, S], F32, isOutput=False)
    sinT = dram("sinT", [128, S], F32, isOutput=False)
    rotT = dram("rotT", [128, 128], MM_DT, isOutput=False)
    outT = dram("outT", [HID, S], F32, isOutput=True)
    if debug:
        d_qlat = dram("d_qlat", [QL, HJ], F32, isOutput=True)
        d_Rq = dram("d_Rq", [128, HJ], F32, isOutput=True)
        d_latn = dram("d_latn", [KVL, HJ], F32, isOutput=True)
        d_kpe = dram("d_kpe", [128, HJ], F32, isOutput=True)
        d_kT = dram("d_kT", [KVL, HJ], F32, isOutput=True)
        d_v = dram("d_v", [KVL, GW], F32, isOutput=True)
        d_qT = dram("d_qT", [768, HJ], F32, isOutput=True)
        d_e = dram("d_e", [128, HJ], F32, isOutput=True)
        d_e1 = dram("d_e1", [128, HJ], F32, isOutput=True)
        d_e3 = dram("d_e3", [128, HJ], F32, isOutput=True)
        d_po = dram("d_po", [128, HJ], F32, isOutput=True)
        d_sum = dram("d_sum", [1, HJ], F32, isOutput=True)
        d_attn = dram("d_attn", [128, HJ], F32, isOutput=True)
        d_attn1 = dram("d_attn1", [128, HJ], F32, isOutput=True)

    AF = mybir.ActivationFunctionType
    MULT = mybir.AluOpType.mult
    ADD = mybir.AluOpType.add

    with tile.TileContext(nc) as tc:
        with (
            tc.tile_pool(name="consts", bufs=1) as consts,
            tc.tile_pool(name="sb", bufs=1) as sb,
            tc.tile_pool(name="ps", space="PSUM", bufs=1) as ps,
        ):
            # ---- constants ----
            ones_f = consts.tile([128, 1], F32, name="ones_f")
            nc.vector.memset(ones_f[:], 1.0)
            ones_r = consts.tile([128, 1], MM_DT, name="ones_r")
            nc.vector.tensor_copy(ones_r[:], ones_f[:])
            eps_sb = consts.tile([128, 1], F32, name="eps_sb")
            nc.vector.memset(eps_sb[:], EPS)
            rot_sb = consts.tile([128, 128], MM_DT, name="rot_sb")
            nc.sync.dma_start(rot_sb[:], rotT[:, :])
            cos_sb = consts.tile([128, S], F32, name="cos_sb")
            sin_sb = consts.tile([128, S], F32, name="sin_sb")
            nc.sync.dma_start(cos_sb[:], cosT[:, :])
            nc.sync.dma_start(sin_sb[:], sinT[:, :])
            # causal mask tiles: cmask[t][p, x] = 1.0 if x - p >= 128*t else 0
            cmask = []
            for t in range(4):
                mt = consts.tile([128, HJ], BF16, name=f"cmask{t}")
                nc.gpsimd.memset(mt[:], 1.0)
                nc.gpsimd.affine_select(
                    out=mt[:], in_=mt[:],
                    compare_op=mybir.AluOpType.is_ge, fill=0.0,
                    base=-128 * t, pattern=[[1, HJ]], channel_multiplier=-1)
                cmask.append(mt)

            # ---- persistent k-side tensors (full S) ----
            latn = [sb.tile([128, S], MM_DT, name=f"latn{m}", tag="latn", bufs=4)
                    for m in range(4)]
            kpeT = sb.tile([128, S], MM_DT, name="kpeT", tag="kpeT", bufs=1)
            kT = [sb.tile([128, S], MM_DT, name=f"kT{hh}", tag="kT", bufs=4)
                  for hh in range(HG)]
            vsb = [sb.tile([128, GW], MM_DT, name=f"v{i}", tag="v", bufs=8)
                   for i in range(8)]

            # kv_b weights, resident
            kbn_sb = []
            kbv_sb = []
            for k4 in range(4):
                tn = sb.tile([128, GW], MM_DT, name=f"kbn{k4}", tag="kbn", bufs=4)
                nc.sync.dma_start(tn[:], wkbn[k4 * 128:(k4 + 1) * 128, :])
                kbn_sb.append(tn)
                tv = sb.tile([128, GW], MM_DT, name=f"kbv{k4}", tag="kbv", bufs=4)
                nc.sync.dma_start(tv[:], wkbv[k4 * 128:(k4 + 1) * 128, :])
                kbv_sb.append(tv)

            wqa_r = wqa.rearrange("(k p) f -> p k f", p=128)
            wkva_r = wkva.rearrange("(k p) f -> p k f", p=128)
            wqb_r = wqb.rearrange("(k p) f -> p k f", p=128)
            wo_r = wo.rearrange("(k p) f -> p k f", p=128)

            for j in range(2):
                jsl = slice(j * HJ, (j + 1) * HJ)

                # ---- hidden half, transposed ----
                ht = []
                for k in range(NKT):
                    t = sb.tile([128, HJ], MM_DT, name=f"ht{j}_{k}", tag="ht", bufs=NKT)
                    nc.sync.dma_start(t[:], hT[k * 128:(k + 1) * 128, jsl])
                    ht.append(t)

                # ---- S1: q_lat^T = q_a_w @ h^T (12 stripes) + sum of squares ----
                ql_t = []
                ps_msq = ps.tile([1, HJ], F32, name=f"msq_q{j}", tag="row", bufs=2)
                for m in range(NQL):
                    wst = sb.tile([128, NKT, 128], MM_DT, name=f"wqa{j}_{m}", tag="wstripe", bufs=2)
                    nc.sync.dma_start(wst[:], wqa_r[:, :, m * 128:(m + 1) * 128])
                    pm = ps.tile([128, HJ], F32, name=f"ps_qa{j}_{m}", tag="mm", bufs=4)
                    for k in range(NKT):
                        nc.tensor.matmul(pm[:], wst[:, k, :], ht[k][:],
                                         start=(k == 0), stop=(k == NKT - 1))
                    qt = sb.tile([128, HJ], MM_DT, name=f"ql{j}_{m}", tag="ql", bufs=NQL)
                    nc.vector.tensor_copy(qt[:], pm[:])
                    sqt = sb.tile([128, HJ], MM_DT, name=f"sq_q{j}_{m}", tag="tmp", bufs=4)
                    nc.scalar.activation(sqt[:], pm[:], AF.Square)
                    nc.tensor.matmul(ps_msq[:], ones_r[:], sqt[:],
                                     start=(m == 0), stop=(m == NQL - 1))
                    if debug and j == 0:
                        nc.sync.dma_start(d_qlat[m * 128:(m + 1) * 128, :], qt[:].bitcast(F32))
                    ql_t.append(qt)

                # rmsnorm scale row for q (applied at the q^T stage)
                sr_q = sb.tile([1, HJ], F32, name=f"sr_q{j}", tag="srow", bufs=2)
                nc.scalar.activation(sr_q[:], ps_msq[:], AF.Sqrt, bias=eps_sb[0:1, :], scale=1.0 / QL)
                rr_q = sb.tile([1, HJ], F32, name=f"rr_q{j}", tag="srow", bufs=2)
                nc.vector.reciprocal(rr_q[:], sr_q[:])
                R_q = sb.tile([128, HJ], F32, name=f"R_q{j}", tag="bcast", bufs=4)
                nc.gpsimd.partition_broadcast(R_q[:], rr_q[:])
                if debug and j == 0:
                    nc.sync.dma_start(d_Rq[:, :], R_q[:])

                # ---- S3: kv_a -> latent (4 stripes) + k_pe (64 rows) ----
                ps_msk = ps.tile([1, HJ], F32, name=f"msq_kv{j}", tag="row", bufs=2)
                for m in range(4):
                    wst = sb.tile([128, NKT, 128], MM_DT, name=f"wkva{j}_{m}", tag="wstripe", bufs=2)
                    nc.sync.dma_start(wst[:], wkva_r[:, :, m * 128:(m + 1) * 128])
                    pm = ps.tile([128, HJ], F32, name=f"ps_kva{j}_{m}", tag="mm", bufs=4)
                    for k in range(NKT):
                        nc.tensor.matmul(pm[:], wst[:, k, :], ht[k][:],
                                         start=(k == 0), stop=(k == NKT - 1))
                    nc.vector.tensor_copy(latn[m][:, jsl], pm[:])
                    sqt = sb.tile([128, HJ], MM_DT, name=f"sq_kv{j}_{m}", tag="tmp", bufs=4)
                    nc.scalar.activation(sqt[:], pm[:], AF.Square)
                    nc.tensor.matmul(ps_msk[:], ones_r[:], sqt[:],
                                     start=(m == 0), stop=(m == 3))
                sr_k = sb.tile([1, HJ], F32, name=f"sr_k{j}", tag="srow", bufs=2)
                nc.scalar.activation(sr_k[:], ps_msk[:], AF.Sqrt, bias=eps_sb[0:1, :], scale=1.0 / KVL)
                rr_k = sb.tile([1, HJ], F32, name=f"rr_k{j}", tag="srow", bufs=2)
                nc.vector.reciprocal(rr_k[:], sr_k[:])
                R_kv = sb.tile([128, HJ], F32, name=f"R_kv{j}", tag="bcast", bufs=4)
                nc.gpsimd.partition_broadcast(R_kv[:], rr_k[:])
                for m in range(4):
                    nc.vector.tensor_tensor(out=latn[m][:, jsl],
                                            in0=latn[m][:, jsl].bitcast(F32),
                                            in1=R_kv[:], op=MULT)

                if debug and j == 0:
                    for m in range(4):
                        nc.sync.dma_start(d_latn[m * 128:(m + 1) * 128, :],
                                          latn[m][:, jsl].bitcast(F32))
                # k_pe stripe: matmul (64-wide) + rope, no norm
                wpe = sb.tile([128, NKT, 128], MM_DT, name=f"wpe{j}", tag="wstripe", bufs=2)
                nc.sync.dma_start(wpe[:], wkva_r[:, :, KVL:KVL + 2 * ROPE])
                pm = ps.tile([128, HJ], F32, name=f"ps_pe{j}", tag="mm", bufs=4)
                for k in range(NKT):
                    nc.tensor.matmul(pm[:], wpe[:, k, :], ht[k][:],
                                     start=(k == 0), stop=(k == NKT - 1))
                xpe = sb.tile([128, HJ], MM_DT, name=f"xpe{j}", tag="tmp", bufs=4)
                nc.vector.tensor_copy(xpe[:], pm[:])
                pr = ps.tile([128, HJ], F32, name=f"ps_rot{j}", tag="mm", bufs=4)
                nc.tensor.matmul(pr[:], rot_sb[:], xpe[:], start=True, stop=True)
                t1 = sb.tile([128, HJ], F32, name=f"t1k{j}", tag="tmp", bufs=4)
                nc.vector.tensor_tensor(out=t1[:], in0=xpe[:].bitcast(F32),
                                        in1=cos_sb[:, jsl], op=MULT)
                t2 = sb.tile([128, HJ], F32, name=f"t2k{j}", tag="tmp", bufs=4)
                nc.vector.tensor_tensor(out=t2[:], in0=pr[:], in1=sin_sb[:, jsl], op=MULT)
                nc.vector.tensor_tensor(out=kpeT[:, jsl], in0=t1[:], in1=t2[:], op=ADD)

                if debug and j == 0:
                    nc.sync.dma_start(d_kpe[:, :], kpeT[:, jsl].bitcast(F32))
                # ---- S4: k_nope^T per head ----
                for hh in range(HG):
                    pm = ps.tile([128, HJ], F32, name=f"ps_kn{j}_{hh}", tag="mm", bufs=4)
                    for k4 in range(4):
                        nc.tensor.matmul(pm[:], kbn_sb[k4][:, hh * 128:(hh + 1) * 128],
                                         latn[k4][:, jsl], start=(k4 == 0), stop=(k4 == 3))
                    nc.scalar.copy(kT[hh][:, jsl], pm[:])

                if debug and j == 0:
                    for hh in range(HG):
                        nc.sync.dma_start(d_kT[hh * 128:(hh + 1) * 128, :],
                                          kT[hh][:, jsl].bitcast(F32))
                # ---- S5: v (natural layout) per 128-token chunk ----
                for tt in range(4):
                    i = 4 * j + tt
                    csl = slice(j * HJ + tt * 128, j * HJ + (tt + 1) * 128)
                    pm = ps.tile([128, GW], F32, name=f"ps_v{i}", tag="mm", bufs=4)
                    for k4 in range(4):
                        nc.tensor.matmul(pm[:], latn[k4][:, csl], kbv_sb[k4][:],
                                         start=(k4 == 0), stop=(k4 == 3))
                    nc.scalar.copy(vsb[i][:], pm[:])

                if debug and j == 0:
                    for tt in range(4):
                        nc.sync.dma_start(d_v[tt * 128:(tt + 1) * 128, :],
                                          vsb[tt][:].bitcast(F32))
                # ---- S2: q^T stripes (4 nope heads + 2 pe pairs) ----
                qT = []
                for m in range(6):
                    wst = sb.tile([128, NQL, 128], MM_DT, name=f"wqb{j}_{m}", tag="wstripe", bufs=2)
                    nc.sync.dma_start(wst[:], wqb_r[:, :, m * 128:(m + 1) * 128])
                    pm = ps.tile([128, HJ], F32, name=f"ps_qb{j}_{m}", tag="mm", bufs=4)
                    for k in range(NQL):
                        nc.tensor.matmul(pm[:], wst[:, k, :], ql_t[k][:],
                                         start=(k == 0), stop=(k == NQL - 1))
                    qt = sb.tile([128, HJ], MM_DT, name=f"qT{j}_{m}", tag="qT", bufs=6)
                    if m < 4:
                        nc.vector.tensor_tensor(out=qt[:], in0=pm[:], in1=R_q[:], op=MULT)
                    else:
                        xq = sb.tile([128, HJ], MM_DT, name=f"xq{j}_{m}", tag="tmp", bufs=4)
                        nc.vector.tensor_copy(xq[:], pm[:])
                        pr = ps.tile([128, HJ], F32, name=f"ps_rotq{j}_{m}", tag="mm", bufs=4)
                        nc.tensor.matmul(pr[:], rot_sb[:], xq[:], start=True, stop=True)
                        t1 = sb.tile([128, HJ], F32, name=f"t1q{j}_{m}", tag="tmp", bufs=4)
                        nc.vector.tensor_tensor(out=t1[:], in0=xq[:].bitcast(F32),
                                                in1=cos_sb[:, jsl], op=MULT)
                        t2 = sb.tile([128, HJ], F32, name=f"t2q{j}_{m}", tag="tmp", bufs=4)
                        nc.vector.tensor_tensor(out=t2[:], in0=pr[:], in1=sin_sb[:, jsl], op=MULT)
                        t3 = sb.tile([128, HJ], F32, name=f"t3q{j}_{m}", tag="tmp", bufs=4)
                        nc.vector.tensor_tensor(out=t3[:], in0=t1[:], in1=t2[:], op=ADD)
                        nc.vector.tensor_tensor(out=qt[:], in0=t3[:], in1=R_q[:], op=MULT)
                    if debug and j == 0:
                        nc.sync.dma_start(d_qT[m * 128:(m + 1) * 128, :], qt[:].bitcast(F32))
                    qT.append(qt)

                # ---- S6/S7: attention per head ----
                attn = []
                for hh in range(HG):
                    qpe = qT[4 + hh // 2][(hh % 2) * 64:(hh % 2) * 64 + 64, :]
                    po = ps.tile([128, HJ], F32, name=f"ps_o{j}_{hh}", tag="acc", bufs=2)
                    psum = ps.tile([1, HJ], F32, name=f"ps_sum{j}_{hh}", tag="row", bufs=2)
                    irange = list(range(4 * (j + 1)))
                    last = irange[-1]
                    for i in irange:
                        pss = ps.tile([128, HJ], F32, name=f"ps_s{j}_{hh}_{i}", tag="mm", bufs=4)
                        nc.tensor.matmul(pss[:], kT[hh][:, i * 128:(i + 1) * 128], qT[hh][:],
                                         start=True, stop=False)
                        pe0 = (hh % 2) * 64
                        nc.tensor.matmul(pss[:], kpeT[pe0:pe0 + 64, i * 128:(i + 1) * 128],
                                         qpe, start=False, stop=True)
                        et = sb.tile([128, HJ], MM_DT, name=f"e{j}_{hh}_{i}", tag="expT", bufs=2)
                        nc.scalar.activation(et[:], pss[:], AF.Exp)
                        if i * 128 + 127 > j * HJ:  # diagonal-crossing tile: causal mask
                            nc.vector.tensor_tensor(out=et[:], in0=et[:].bitcast(F32),
                                                    in1=cmask[i - 4 * j][:], op=MULT)
                        if debug and j == 0 and hh == 0 and i == 0:
                            nc.sync.dma_start(d_e[:, :], et[:].bitcast(F32))
                        if debug and j == 0 and hh == 0 and i == 1:
                            nc.sync.dma_start(d_e1[:, :], et[:].bitcast(F32))
                        if debug and j == 0 and hh == 0 and i == 3:
                            nc.sync.dma_start(d_e3[:, :], et[:].bitcast(F32))
                        nc.tensor.matmul(psum[:], ones_r[:], et[:],
                                         start=(i == 0), stop=(i == last))
                        nc.tensor.matmul(po[:], vsb[i][:, hh * 128:(hh + 1) * 128], et[:],
                                         start=(i == 0), stop=(i == last))
                    if debug and j == 0 and hh == 0:
                        dtmp = sb.tile([128, HJ], F32, name="dtmp_po", tag="bcast", bufs=4)
                        nc.scalar.copy(dtmp[:], po[:])
                        nc.sync.dma_start(d_po[:, :], dtmp[:])
                        dtmp2 = sb.tile([1, HJ], F32, name="dtmp_sum", tag="srow", bufs=2)
                        nc.scalar.copy(dtmp2[:], psum[:])
                        nc.sync.dma_start(d_sum[:, :], dtmp2[:])
                    rs = sb.tile([1, HJ], F32, name=f"rs{j}_{hh}", tag="srow", bufs=2)
                    nc.vector.reciprocal(rs[:], psum[:])
                    Rs = sb.tile([128, HJ], F32, name=f"Rs{j}_{hh}", tag="bcast", bufs=4)
                    nc.gpsimd.partition_broadcast(Rs[:], rs[:])
                    at = sb.tile([128, HJ], MM_DT, name=f"attn{j}_{hh}", tag="attn", bufs=4)
                    nc.vector.tensor_tensor(out=at[:], in0=po[:], in1=Rs[:], op=MULT)
                    if debug and j == 0 and hh == 0:
                        nc.sync.dma_start(d_attn[:, :], at[:].bitcast(F32))
                    if debug and j == 0 and hh == 1:
                        nc.sync.dma_start(d_attn1[:, :], at[:].bitcast(F32))
                    attn.append(at)

                # ---- S8: o_proj transposed: outT[o, t] = sum_c wo[c, o] attn_catT[c, t] ----
                for oc in range(NKT):
                    wos = sb.tile([128, HG, 128], MM_DT, name=f"wo{j}_{oc}", tag="wos", bufs=2)
                    nc.sync.dma_start(wos[:], wo_r[:, :, oc * 128:(oc + 1) * 128])
                    pm = ps.tile([128, HJ], F32, name=f"ps_out{j}_{oc}", tag="mm", bufs=4)
                    for hh in range(HG):
                        nc.tensor.matmul(pm[:], wos[:, hh, :], attn[hh][:],
                                         start=(hh == 0), stop=(hh == HG - 1))
                    ot = sb.tile([128, HJ], F32, name=f"ot{j}_{oc}", tag="osb", bufs=2)
                    nc.vector.tensor_copy(ot[:], pm[:])
                    nc.sync.dma_start(outT[oc * 128:(oc + 1) * 128, jsl], ot[:])

    nc.compile()
    return nc


_NC = None


def _get_nc():
    global _NC
    if _NC is None:
        _NC = build_nc()
    return _NC


def run(inputs, trace=False):
    in_maps = prep_in_maps(inputs)
    nc = _get_nc()
    res = run_bass_kernel_spmd(nc, in_maps, core_ids=list(range(8)), trace=trace)
    out = combine_outputs(res.results)
    return out, res


def kernel(**inputs):
    out, _ = run(inputs)
    return out.astype(np.float32)

